# revision 1
# baseline (speedup 1.0000x reference)
"""W4A16 group-quantized linear (CudaW4A16Linear) on 8 TRN2 NeuronCores.

Column-parallel sharding: out_features O=11008 split across 8 cores
(1376 rows each); x replicated; per-core output [64, 1376] f32 gathered
on host.

Per-core dataflow:
  - qweight shard repacked on host to u16-transposed layout
    qt[8, 128, OC]: partition p of word-tile t is u16 word w = 128*t + p,
    holding int4 nibbles k = 4w .. 4w+3 (LSB-first).
  - DVE scalar_tensor_tensor:  w_j = (qt & (0xF << 4j)) * s_bcast
    -> fp16 value  n_j * 16^j * s[g,o]  in matmul-ready [k-part, o-free]
    layout.  One DVE op does unpack+scale; the 16^-j shift is folded into
    host-prescaled x (exact powers of two), with a global C=64 prescale to
    keep x*64/16^j away from fp16 subnormals.
  - PE: psum[64, Nc] += xt4[j,t].T @ w_j  (32 matmuls), plus one K=32
    correction matmul  -C * t_g.T @ (z*s)  that removes the zero-points
    (t_g = per-group sums of x).
  - Evacuate psum * (1/C) -> fp32 out.
"""

import sys

sys.path.insert(0, "/opt/trn_rl_repo")

import numpy as np

import concourse.bass as bass
import concourse.bacc as bacc
import concourse.mybir as mybir
import concourse.tile as tile
from concourse.bass_utils import run_bass_kernel_spmd

GROUP = 128
K = 4096
O = 11008
M = 64
NCORES = 8
OC = O // NCORES  # 1376
NT = K // 4 // 128  # 8 u16-word tiles per core
G = K // GROUP  # 32 groups
CSCALE = 64.0  # global x prescale (power of 2)
F16 = mybir.dt.float16
F32 = mybir.dt.float32
U16 = mybir.dt.uint16

# o-chunks for PSUM banks (free dim <= 512 fp32)
CHUNKS = [(0, 512), (512, 512), (1024, 352)]


def build_bass(mode="mod"):
    """mode: 'mod'  -> one DVE STT per (t,j): w'_j = (q mod 16^{j+1}) * s_b
                      (junk below nibble j cancels via host x-coefficients)
             'two'  -> TS(and mask_j) -> fp16, then TT(* s_b)."""
    nc = bacc.Bacc()
    qt = nc.declare_dram_parameter("qt", [NT, 128, OC], U16, isOutput=False)
    xt4 = nc.declare_dram_parameter("xt4", [128, 4, NT, M], F16, isOutput=False)
    if mode == "safe":
        sbx = nc.declare_dram_parameter("sbx", [NT, 128, OC], F16, isOutput=False)
    else:
        st = nc.declare_dram_parameter("st", [G, OC], F16, isOutput=False)
    zst = nc.declare_dram_parameter("zst", [G, OC], F16, isOutput=False)
    ttn = nc.declare_dram_parameter("ttn", [G, M], F16, isOutput=False)
    out = nc.declare_dram_parameter("out", [M, OC], F32, isOutput=True)

    with tile.TileContext(nc) as tc:
        with (
            tc.tile_pool(name="const", bufs=1) as cpool,
            tc.tile_pool(name="q", bufs=2) as qpool,
            tc.tile_pool(name="w", bufs=4) as wpool,
            tc.tile_pool(name="psum", bufs=1, space="PSUM") as ppool,
            tc.tile_pool(name="o", bufs=1) as opool,
        ):
            # ---- persistent constants ----
            xt_sb = cpool.tile([128, 4, NT, M], F16, tag="xt")
            nc.sync.dma_start(xt_sb[:], xt4[:])
            ttn_sb = cpool.tile([G, M], F16, tag="ttn")
            nc.sync.dma_start(ttn_sb[:], ttn[:])
            zst_sb = cpool.tile([G, OC], F16, tag="zst")
            nc.sync.dma_start(zst_sb[:], zst[:])


            # scale broadcast tiles: sb_t[p, :] = st[4t + p//32, :]
            sbt = []
            for t in range(NT):
                sb = cpool.tile([128, OC], F16, tag=f"sb{t}", name=f"sb{t}")
                if mode == "safe":
                    nc.sync.dma_start(sb[:], sbx[t])
                else:
                    for gi in range(4):
                        nc.sync.dma_start(
                            sb[32 * gi : 32 * (gi + 1), :],
                            st[4 * t + gi : 4 * t + gi + 1, :].broadcast_to([32, OC]),
                        )
                sbt.append(sb)

            # ---- PSUM accumulators, one per o-chunk ----
            psums = [
                ppool.tile([M, n], F32, tag=f"ps{ci}", name=f"ps{ci}")
                for ci, (_, n) in enumerate(CHUNKS)
            ]

            # ---- main loop ----
            for t in range(NT):
                qtile = qpool.tile([128, OC], U16, tag="q")
                nc.sync.dma_start(qtile[:], qt[t])
                for j in range(4):
                    w = wpool.tile([128, OC], F16, tag="w")
                    if mode == "mod":
                        nc.vector.scalar_tensor_tensor(
                            w[:],
                            qtile[:],
                            float(16 ** (j + 1)),
                            sbt[t][:],
                            op0=mybir.AluOpType.mod,
                            op1=mybir.AluOpType.mult,
                        )
                    elif mode == "two":
                        nib = wpool.tile([128, OC], U16, tag="nib")
                        nc.vector.tensor_scalar(
                            nib[:],
                            qtile[:],
                            15 << (4 * j),
                            None,
                            op0=mybir.AluOpType.bitwise_and,
                        )
                        nc.vector.tensor_tensor(
                            w[:], nib[:], sbt[t][:], op=mybir.AluOpType.mult
                        )
                    else:  # safe: and -> copy-cast -> mult, all same-dtype ops
                        nib = wpool.tile([128, OC], U16, tag="nib")
                        nc.vector.tensor_scalar(
                            nib[:],
                            qtile[:],
                            15 << (4 * j),
                            None,
                            op0=mybir.AluOpType.bitwise_and,
                        )
                        nibf = wpool.tile([128, OC], F16, tag="nibf")
                        nc.vector.tensor_copy(nibf[:], nib[:])
                        nc.vector.tensor_tensor(
                            w[:], nibf[:], sbt[t][:], op=mybir.AluOpType.mult
                        )
                    for ci, (c0, n) in enumerate(CHUNKS):
                        nc.tensor.matmul(
                            psums[ci][:],
                            xt_sb[:, j, t, :],
                            w[:, c0 : c0 + n],
                            start=(t == 0 and j == 0),
                            stop=False,
                        )

            # ---- zero-point correction + evacuation ----
            out_sb = opool.tile([M, OC], F32, tag="out")
            for ci, (c0, n) in enumerate(CHUNKS):
                nc.tensor.matmul(
                    psums[ci][:],
                    ttn_sb[:],
                    zst_sb[:, c0 : c0 + n],
                    start=False,
                    stop=True,
                )
                nc.vector.tensor_scalar_mul(
                    out_sb[:, c0 : c0 + n], psums[ci][:], 1.0 / CSCALE
                )
            nc.sync.dma_start(out[:], out_sb[:])

    nc.finalize()
    return nc


def prep_host(x, qweight_i32, qzeros_i32, scales_f16, mode="mod"):
    """Build per-core input maps. Shapes: x [4,16,4096] f16,
    qweight [11008,512] i32, qzeros [11008,4] i32, scales [11008,32] f16."""
    x2 = np.asarray(x, dtype=np.float16).reshape(-1, K)  # [64, 4096]
    assert x2.shape == (M, K)

    # xt4[p, j, t, m]: coefficient applied to the j-th nibble plane of
    # u16 word w = 128t + p (which holds k = 4w .. 4w+3).
    xr = x2.reshape(M, NT, 128, 4)  # [m, t, p, j]
    xt4 = np.transpose(xr, (2, 3, 1, 0)).astype(np.float32)  # [p, j, t, m]
    if mode == "mod":
        # w'_j = s*(q mod 16^{j+1});  sum_j x_j*s*n_j
        #      = sum_j w'_j * 16^-j * (x_j - x_{j+1}/16)
        xc = xt4.copy()
        xc[:, :3] -= xt4[:, 1:] / 16.0
        xt4 = xc
    for j in range(4):
        xt4[:, j] *= CSCALE / (16.0**j)
    xt4 = np.ascontiguousarray(xt4.astype(np.float16))

    # per-group sums of x, negated and prescaled for the correction matmul
    tg = x2.astype(np.float32).reshape(M, G, GROUP).sum(axis=2)  # [64, 32]
    ttn = np.ascontiguousarray((-CSCALE * tg.T).astype(np.float16))  # [32, 64]

    qw = np.ascontiguousarray(np.asarray(qweight_i32, dtype=np.int32))
    qz = np.ascontiguousarray(np.asarray(qzeros_i32, dtype=np.int32)).view(np.uint32)
    sc = np.asarray(scales_f16, dtype=np.float16)

    in_maps = []
    for c in range(NCORES):
        o0, o1 = c * OC, (c + 1) * OC
        # u16-transposed packed weights: [K/4, OC] -> [NT, 128, OC]
        qtc = (
            qw[o0:o1]
            .view(np.uint16)
            .T.reshape(NT, 128, OC)
        )
        qtc = np.ascontiguousarray(qtc)

        # unpack zeros on host: z[o, g]
        gidx = np.arange(G)
        z = (qz[o0:o1, gidx // 8] >> (4 * (gidx % 8))[None, :]) & 15  # [OC, G]
        s32 = sc[o0:o1].astype(np.float32)  # [OC, G]
        zst = np.ascontiguousarray((z.astype(np.float32) * s32).T.astype(np.float16))
        stc = np.ascontiguousarray(sc[o0:o1].T)  # [G, OC] f16

        m = {"qt": qtc, "xt4": xt4, "zst": zst, "ttn": ttn}
        if mode == "safe":
            # host-expanded scale broadcast: sbx[t, p, :] = stc[4t + p//32, :]
            m["sbx"] = np.ascontiguousarray(
                np.repeat(stc, 32, axis=0).reshape(NT, 128, OC)
            )
        else:
            m["st"] = stc
        in_maps.append(m)
    return in_maps


_NC_CACHE = {}


def kernel(x, qweight_i32, qzeros_i32, scales_f16, _trace=False, _tmpdir=None, _mode="safe"):
    in_maps = prep_host(x, qweight_i32, qzeros_i32, scales_f16, mode=_mode)
    if _mode not in _NC_CACHE:
        _NC_CACHE[_mode] = build_bass(mode=_mode)
    nc = _NC_CACHE[_mode]
    res = run_bass_kernel_spmd(
        nc,
        in_maps,
        core_ids=list(range(NCORES)),
        trace=_trace,
        tmpdir=_tmpdir,
    )
    outs = [res.results[c]["out"] for c in range(NCORES)]
    full = np.concatenate(outs, axis=1).astype(np.float32)  # [64, 11008]
    out = full.reshape(4, 16, O)
    if _trace:
        kernel.last_exec_time_ns = res.exec_time_ns
        kernel.last_results = res
    return out



# revision 33
# speedup vs baseline: 6.6075x; 6.6075x over previous
"""W4A16 group-quantized linear (CudaW4A16Linear) on 8 TRN2 NeuronCores.

Column-parallel sharding: out_features O=11008 split across 8 cores
(OC=1376 rows each); x replicated; per-core output [64, 1376] f32
gathered on host.

Per-core dataflow (v2):
  - qweight shard repacked on host to u16-transposed layout
    qt[NT=8, 128, OC]: partition p of word-tile t is u16 word w = 128*t+p,
    holding int4 nibbles k = 4w .. 4w+3 (LSB-first).
  - sbx[NT, 128, OC] f16: host-expanded per-group scales s[g,o] broadcast
    to the word-tile layout (sbx[t, p, :] = s[4t + p//32, :]).
  - All inputs DMA'd once into persistent SBUF tiles; DMA issue is spread
    across the SP / Activation / GPSIMD sequencers so no single sequencer
    serializes the loads.
  - Dequant per (t, j) plane: nib = qt & (0xF << 4j) on DVE (4x mode),
    then w = nib * sbx on DVE (2x) or GPSIMD (static ~1/3 offload) --
    w holds n_j * 16^j * s in matmul-ready [k-part, o-free] layout.
  - PE: psum[64, Nc] += xt4[j,t].T @ w (32 planes x 3 o-chunks), plus one
    K=32 correction matmul -C * t_g.T @ (z*s) per chunk that removes the
    zero-points (t_g = per-group sums of x).
  - Per chunk: correction right after its last main matmul, evacuation
    psum * (1/C) -> f32 on the Activation engine, then the output-column
    DMA -- so the tail overlaps the remaining chunks' matmuls.
"""

import sys

sys.path.insert(0, "/opt/trn_rl_repo")

import numpy as np

import concourse.bass as bass
import concourse.bacc as bacc
import concourse.mybir as mybir
import concourse.tile as tile
from concourse.bass_utils import run_bass_kernel_spmd

GROUP = 128
K = 4096
O = 11008
M = 64
NCORES = 8
OC = O // NCORES  # 1376
NT = K // 4 // 128  # 8 u16-word tiles per core
G = K // GROUP  # 32 groups
CSCALE = 64.0  # global x prescale (power of 2)
F16 = mybir.dt.float16
F32 = mybir.dt.float32
U16 = mybir.dt.uint16

# o-chunks for PSUM banks (moving free dim <= 512 fp32)
CHUNKS = [(0, 512), (512, 512), (1024, 352)]

# planes (i = 4t + j) whose scale-mult runs on GPSIMD instead of DVE.
# The j=3 planes are maskless (raw q*s), so on Pool they gate only on the
# qt DMA -- Pool never waits for DVE. Planes 1,2 (t=0, masked) top Pool up
# early; none in the last tile so the tail stays on the faster DVE.
POOL_PLANES = frozenset({1, 2} | {4 * t + 3 for t in range(7)})

# per-tile plane emission order: pool planes first so Pool/DVE overlap
PLANE_ORDER = {t: [3, 0, 1, 2] for t in range(NT)}
PLANE_ORDER[0] = [1, 2, 3, 0]


def build_bass_v2(pool_planes=POOL_PLANES, reps=1, pool_stt=False):
    """reps>1 unrolls the whole kernel body (including input DMAs) N times
    in one program -- used only for slope-based wall-clock timing.
    pool_stt: Pool planes use a single fused (q & mask) * s
    scalar_tensor_tensor instead of a DVE mask + Pool tensor_tensor."""
    nc = bacc.Bacc()
    qt = nc.declare_dram_parameter("qt", [NT, 128, OC], U16, isOutput=False)
    sbx = nc.declare_dram_parameter("sbx", [NT, 128, OC], F16, isOutput=False)
    xt4 = nc.declare_dram_parameter("xt4", [128, 4, NT, M], F16, isOutput=False)
    zst = nc.declare_dram_parameter("zst", [G, OC], F16, isOutput=False)
    ttn = nc.declare_dram_parameter("ttn", [G, M], F16, isOutput=False)
    out = nc.declare_dram_parameter("out", [M, OC], F32, isOutput=True)

    # last plane in emission order: the per-chunk evac + store hang off it
    last_plane = 4 * (NT - 1) + PLANE_ORDER[NT - 1][-1]

    with tile.TileContext(nc) as tc:
        with (
            tc.tile_pool(name="const", bufs=1 if reps == 1 else 2) as cpool,
            tc.tile_pool(name="nib", bufs=10) as npool,
            tc.tile_pool(name="w", bufs=14) as wpool,
            tc.tile_pool(name="psum", bufs=1, space="PSUM") as ppool,
            tc.tile_pool(name="o", bufs=1 if reps == 1 else 2) as opool,
        ):
          for _rep in range(reps):
            # ---- persistent inputs, loaded once ----
            qt_sb = cpool.tile([128, NT, OC], U16, tag="qt")
            sbx_sb = cpool.tile([128, NT, OC], F16, tag="sbx")
            xt_sb = cpool.tile([128, 4, NT, M], F16, tag="xt")
            ttn_sb = cpool.tile([G, M], F16, tag="ttn")
            zst_sb = cpool.tile([G, OC], F16, tag="zst")

            # interleave issue so tile t=0 of both streams lands first
            # (qt0 before anything else -- it gates the first mask);
            # qt + small tensors on SP, sbx on Activation (Pool stays free
            # for its share of the scale-mults)
            nc.gpsimd.dma_start(ttn_sb[:], ttn[:])
            nc.gpsimd.dma_start(zst_sb[:], zst[:])
            nc.sync.dma_start(qt_sb[:, 0, :], qt[0])
            nc.scalar.dma_start(sbx_sb[:, 0, :], sbx[0])
            nc.sync.dma_start(qt_sb[:, 1, :], qt[1])
            nc.scalar.dma_start(sbx_sb[:, 1, :], sbx[1])
            nc.sync.dma_start(qt_sb[:, 2, :], qt[2])
            nc.scalar.dma_start(xt_sb[:], xt4[:])
            nc.scalar.dma_start(sbx_sb[:, 2, :], sbx[2])
            for t in range(3, NT):
                nc.sync.dma_start(qt_sb[:, t, :], qt[t])
                nc.scalar.dma_start(sbx_sb[:, t, :], sbx[t])

            # ---- PSUM accumulators, one per o-chunk ----
            psums = [
                ppool.tile([M, n], F32, tag=f"ps{ci}", name=f"ps{ci}_r{_rep}")
                for ci, (_, n) in enumerate(CHUNKS)
            ]

            out_sb = opool.tile([M, OC], F32, tag="out")

            # ---- zero-point corrections first: they only need ttn/zst, so
            # running them as the psum-group openers keeps them off the tail
            for ci, (c0, n) in enumerate(CHUNKS):
                nc.tensor.matmul(
                    psums[ci][:],
                    ttn_sb[:],
                    zst_sb[:, c0 : c0 + n],
                    start=True,
                    stop=False,
                )

            # ---- main loop ----
            for t in range(NT):
                for j in PLANE_ORDER[t]:
                    i = 4 * t + j
                    eng = nc.gpsimd if i in pool_planes else nc.vector
                    w = wpool.tile([128, OC], F16, tag="w")
                    if j < 3 and i in pool_planes and pool_stt:
                        # fused (q & mask) * s in one Pool op
                        nc.gpsimd.scalar_tensor_tensor(
                            w[:],
                            qt_sb[:, t, :],
                            15 << (4 * j),
                            sbx_sb[:, t, :],
                            op0=mybir.AluOpType.bitwise_and,
                            op1=mybir.AluOpType.mult,
                        )
                    else:
                        if j == 3:
                            # top plane is raw q * s -- the sub-nibble junk
                            # is cancelled by the adjusted host coefficients
                            # (xt4[:, :3] -= xt4[:, 3:]): one mask saved
                            # per tile
                            src = qt_sb[:, t, :]
                        else:
                            nib = npool.tile([128, OC], U16, tag="nib")
                            nc.vector.tensor_scalar(
                                nib[:],
                                qt_sb[:, t, :],
                                15 << (4 * j),
                                None,
                                op0=mybir.AluOpType.bitwise_and,
                            )
                            src = nib[:]
                        eng.tensor_tensor(
                            w[:], src, sbx_sb[:, t, :], op=mybir.AluOpType.mult
                        )
                    for ci, (c0, n) in enumerate(CHUNKS):
                        nc.tensor.matmul(
                            psums[ci][:],
                            xt_sb[:, j, t, :],
                            w[:, c0 : c0 + n],
                            start=False,
                            stop=(i == last_plane),
                        )
                        if i == last_plane:
                            # evac + store per chunk on one engine each
                            # (Act / DVE / Pool), overlapping the remaining
                            # chunks' final matmuls with no cross-engine
                            # sem hop between evac and store
                            if ci == 1:
                                nc.vector.tensor_scalar_mul(
                                    out_sb[:, c0 : c0 + n],
                                    psums[ci][:],
                                    1.0 / CSCALE,
                                )
                                nc.sync.dma_start(
                                    out[:, c0 : c0 + n], out_sb[:, c0 : c0 + n]
                                )
                            else:
                                nc.scalar.activation(
                                    out_sb[:, c0 : c0 + n],
                                    psums[ci][:],
                                    mybir.ActivationFunctionType.Copy,
                                    scale=1.0 / CSCALE,
                                )
                                nc.scalar.dma_start(
                                    out[:, c0 : c0 + n], out_sb[:, c0 : c0 + n]
                                )

    nc.finalize()
    return nc


def prep_host(x, qweight_i32, qzeros_i32, scales_f16):
    """Build per-core input maps. Shapes: x [4,16,4096] f16,
    qweight [11008,512] i32, qzeros [11008,4] i32, scales [11008,32] f16."""
    x2 = np.asarray(x, dtype=np.float16).reshape(-1, K)  # [64, 4096]
    assert x2.shape == (M, K)

    # xt4[p, j, t, m]: coefficient applied to the j-th nibble plane of
    # u16 word w = 128t + p (which holds k = 4w .. 4w+3).
    xr = x2.reshape(M, NT, 128, 4)  # [m, t, p, j]
    xt4 = np.transpose(xr, (2, 3, 1, 0)).astype(np.float32)  # [p, j, t, m]
    for j in range(4):
        xt4[:, j] *= CSCALE / (16.0**j)
    # the j=3 plane is raw q*s (no mask), which leaks the lower nibbles
    # scaled by 16^{j-3}; cancel them in the lower planes' coefficients
    xt4[:, :3] -= xt4[:, 3:4]
    xt4 = np.ascontiguousarray(xt4.astype(np.float16))

    # per-group sums of x, negated and prescaled for the correction matmul
    tg = x2.astype(np.float32).reshape(M, G, GROUP).sum(axis=2)  # [64, 32]
    ttn = np.ascontiguousarray((-CSCALE * tg.T).astype(np.float16))  # [32, 64]

    qw = np.ascontiguousarray(np.asarray(qweight_i32, dtype=np.int32))
    qz = np.ascontiguousarray(np.asarray(qzeros_i32, dtype=np.int32)).view(np.uint32)
    sc = np.asarray(scales_f16, dtype=np.float16)

    in_maps = []
    for c in range(NCORES):
        o0, o1 = c * OC, (c + 1) * OC
        # u16-transposed packed weights: [K/4, OC] -> [NT, 128, OC]
        qtc = np.ascontiguousarray(qw[o0:o1].view(np.uint16).T.reshape(NT, 128, OC))

        # unpack zeros on host: z[o, g]
        gidx = np.arange(G)
        z = (qz[o0:o1, gidx // 8] >> (4 * (gidx % 8))[None, :]) & 15  # [OC, G]
        s32 = sc[o0:o1].astype(np.float32)  # [OC, G]
        zst = np.ascontiguousarray((z.astype(np.float32) * s32).T.astype(np.float16))
        stc = sc[o0:o1].T  # [G, OC] f16

        # host-expanded scale broadcast: sbx[t, p, :] = stc[4t + p//32, :]
        sbx = np.ascontiguousarray(np.repeat(stc, 32, axis=0).reshape(NT, 128, OC))

        in_maps.append({"qt": qtc, "xt4": xt4, "zst": zst, "ttn": ttn, "sbx": sbx})
    return in_maps


_NC_CACHE = {}


def kernel(x, qweight_i32, qzeros_i32, scales_f16, _trace=False, _tmpdir=None):
    in_maps = prep_host(x, qweight_i32, qzeros_i32, scales_f16)
    if "v2" not in _NC_CACHE:
        _NC_CACHE["v2"] = build_bass_v2()
    nc = _NC_CACHE["v2"]
    res = run_bass_kernel_spmd(
        nc,
        in_maps,
        core_ids=list(range(NCORES)),
        trace=_trace,
        tmpdir=_tmpdir,
    )
    outs = [res.results[c]["out"] for c in range(NCORES)]
    full = np.concatenate(outs, axis=1).astype(np.float32)  # [64, 11008]
    out = full.reshape(4, 16, O)
    if _trace:
        kernel.last_exec_time_ns = res.exec_time_ns
        kernel.last_results = res
    return out
